# revision 1
# baseline (speedup 1.0000x reference)
"""Trainium2 Bass kernel for nn_Aggregate (gnn_message_passing).

Sharding: 8 cores = 2 directions x 4 batch-groups. Cores 0-3 compute
refined_async (source = sync_fea, adj = sync_adj, weights a_*) for 8
batches each; cores 4-7 compute refined_sync. The feature passthrough
(output channels 512:1024) and the no-neighbor fallback select are pure
input data movement, done host-side during unsharding.

Device algorithm per core (8 batches, one direction), all fp32:
  Pair-stack batches on partitions (rows 0-47 and 64-111) so elementwise
  ops run near-full-width. Activations stay feature-major ([feat, node])
  so chained Linears need no transposes; weights are host-pre-transposed
  to input-major so they serve directly as matmul lhsT.

    qT = WqT_s^T @ xT + bq_s        (scaled by 1/sqrt(dh) host-side)
    kT = WkT^T @ xT + bk
    v  = xT^T @ WvT + bv            (node-major; lhsT = xT chunk)
    per (batch, head):  Pq = exp(qT_h^T kT_h)   [q,k]
                        Pk = exp(kT_h^T qT_h)   [k,q]
                        den = Pk^T Af           [q,t]
                        w   = Af / den          [q,t]
                        ST  = Pq^T w            [k,t]
                        SmT = ST * Af           [k,t]
                        GT_h = v_h^T SmT        [d,t]  (pooled, pre-Wo)
    P1 = WoT^T @ G ; M2 = WmT^T @ P1
    cnt = ones^T @ Af ; r = 1/cnt ; out = M2*r^2 + (Wm@bo)*r + bm

  This uses sum_q m*(attn@v@Wo^T + bo) = (sum_q m*attn@v)@Wo^T + cnt*bo,
  and column scaling by r/r^2 commuting through the Wm contraction.

Built on bacc.Bacc: its compile() legalizes sync waits (TRN2 allows one
wait per instruction) via ldweights-wait motion + event semaphores.
"""

import numpy as np

FEA, H, B, N = 256, 8, 32, 48
DH = FEA // H
NB = 8            # batches per core
NPAIR = NB // 2
NCORES = 8

_cached = None


def _finish(nc, out_d, tile_ap, NT):
    ap = tile_ap[:, :, :].rearrange("p a t -> p (a t)")
    for f0 in range(0, 2 * NT, 96):
        nc.sync.dma_start(out=out_d.ap()[:, f0:f0 + 96], in_=ap[:, f0:f0 + 96])


class _Stop(Exception):
    pass


def _build_program(sim_mode=False, phase_limit=7):
    """sim_mode=True memsets psum tiles after allocation so CoreSim's
    initialized-memory tracking passes; the real program never reads the
    pad partitions (48-63, 112-127) of those tiles, so HW skips this."""
    import concourse.tile as tile
    from concourse.tile import add_dep_helper
    from concourse import bacc, mybir
    from contextlib import ExitStack

    f32 = mybir.dt.float32
    bf = mybir.dt.bfloat16
    AF = mybir.ActivationFunctionType
    OP = mybir.AluOpType

    nc = bacc.Bacc("TRN2", target_bir_lowering=False, debug=False)

    # ---- DRAM I/O ----
    hot_d = nc.dram_tensor("hot", [FEA, NB * N + 3 * FEA], bf, kind="ExternalInput")
    adjst_d = nc.dram_tensor("adj_st", [128, NPAIR * N], bf, kind="ExternalInput")
    adjfl_d = nc.dram_tensor("adj_flat", [N, NB * N], bf, kind="ExternalInput")
    w_d = {"womT": nc.dram_tensor("womT", [FEA, FEA], bf, kind="ExternalInput")}
    consts_d = nc.dram_tensor("consts", [128, 8 + FEA], f32, kind="ExternalInput")
    out_d = nc.dram_tensor("outT", [128, 2 * NB * N], bf, kind="ExternalOutput")

    NT = NB * N            # 384

    with ExitStack() as ctx:
      try:
        tc = ctx.enter_context(tile.TileContext(nc))
        sb = ctx.enter_context(tc.tile_pool(name="sb", bufs=1))
        ps = ctx.enter_context(tc.tile_pool(name="ps", bufs=4, space="PSUM"))

        # Each dma_start costs ~0.6us of issue time on its engine's queue;
        # rotate across engines that are idle during the load phase.
        _dma_engines = [nc.sync, nc.gpsimd, nc.scalar]
        _dma_rr = [0]

        def dma(out, in_):
            eng = _dma_engines[_dma_rr[0] % len(_dma_engines)]
            _dma_rr[0] += 1
            eng.dma_start(out=out, in_=in_)

        def ldsplit(dst, src_ap, cols, piece):
            for f0 in range(0, cols, piece):
                f1 = min(f0 + piece, cols)
                dma(dst[:, f0:f1], src_ap[:, f0:f1])

        # ---- loads: emission order ~ need order; early tensors split into
        # small pieces so they spread across DMA queues and arrive fast ----
        # hot blob per K-chunk: [ xT | wqT | wkT | wvT ] in one DMA so the
        # projection matmuls start ~2us in instead of ~6us.
        HOTC = NT + 3 * FEA
        xT, W = [], {"wqT": [], "wkT": [], "wvT": []}
        for kc in range(2):
            t = sb.tile([128, HOTC], bf, tag=f"hot{kc}")
            nc.sync.dma_start(out=t[:, :], in_=hot_d.ap()[kc * 128:(kc + 1) * 128, :])
            xT.append(t[:, 0:NT])
            W["wqT"].append(t[:, NT:NT + FEA])
            W["wkT"].append(t[:, NT + FEA:NT + 2 * FEA])
            W["wvT"].append(t[:, NT + 2 * FEA:NT + 3 * FEA])
        for kc in range(2):
            t = sb.tile([128, FEA], bf, tag=f"womT{kc}")
            ldsplit(t, w_d["womT"].ap()[kc * 128:(kc + 1) * 128, :], FEA, piece=FEA)
            W.setdefault("womT", []).append(t)
        consts = sb.tile([128, 8 + FEA], f32, tag="consts")
        nc.sync.dma_start(out=consts[:, :], in_=consts_d.ap()[:, :])
        adjst = sb.tile([128, NPAIR * N], bf, tag="adjst")
        dma(adjst[:, :], adjst_d.ap()[:, :])
        adjfl = sb.tile([N, NT], bf, tag="adjfl")
        dma(adjfl[:, :], adjfl_d.ap()[:, :])
        bq, bk = consts[:, 0:2], consts[:, 2:4]
        c0, bm = consts[:, 4:6], consts[:, 6:8]
        bvr = consts[:, 8:8 + FEA]

        ones = sb.tile([N, 128], bf, tag="ones")
        nc.vector.memset(ones[:, :], 1.0)

        def pstile():
            t = ps.tile([128, 2, 512], f32, tag="ps")
            if sim_mode:
                nc.vector.memset(t[:, :, :], 1.0)
            return t

        # ---- q/k projections (feature-major, all 8 batches batched) ----
        qkT = {}
        for nm, bias, wnm in (("q", bq, "wqT"), ("k", bk, "wkT")):
            p = pstile()
            for ot in range(2):
                for kc in range(2):
                    nc.tensor.matmul(
                        p[:, ot, 0:NT],
                        W[wnm][kc][:, ot * 128:(ot + 1) * 128],
                        xT[kc][:, :],
                        start=(kc == 0), stop=(kc == 1),
                    )
            dst = sb.tile([128, 2, NT], bf, tag=f"{nm}T")
            nc.vector.tensor_tensor(
                out=dst[:, :, :], in0=p[:, :, 0:NT],
                in1=bias[:, :, None].to_broadcast((128, 2, NT)),
                op=OP.add,
            )
            qkT[nm] = dst
        qT, kT = qkT["q"], qkT["k"]

        # ---- v (node-major: [k at j*64, pair*256 + o]) ----
        if phase_limit < 2:
            _finish(nc, out_d, qT, NT); raise _Stop
        vp = pstile()
        for b in range(NB):
            pr, j = b // 2, b % 2
            bank, off = pr // 2, (pr % 2) * FEA
            for kc in range(2):
                nc.tensor.matmul(
                    vp[j * 64:j * 64 + N, bank, off:off + FEA],
                    xT[kc][:, b * N:(b + 1) * N],
                    W["wvT"][kc][:, :],
                    start=(kc == 0), stop=(kc == 1),
                )
        v = sb.tile([128, 2, 2, FEA], bf, tag="v")
        nc.vector.tensor_tensor(
            out=v[:, :, :, :],
            in0=vp[:, :, :].rearrange("p a (c o) -> p a c o", o=FEA),
            in1=bvr[:, None, None, :].to_broadcast((128, 2, 2, FEA)),
            op=OP.add,
        )

        def head_slice(t, h, b):
            """[32, 48] slice of a feature-major [128, 2, NT] tile."""
            return t[(h % 4) * 32:(h % 4) * 32 + 32, h // 4, b * N:(b + 1) * N]

        # ---- scores, both orientations, exp ----
        # Row-tiled matmuls must not write the same PSUM bank concurrently
        # (HW constraint). Map row-group -> bank bijectively: tile t's bank b
        # holds head-group g = 2t+b (heads {g, g+4}), pairs in the free dim.
        if phase_limit < 3:
            _finish(nc, out_d, qT, NT); raise _Stop

        def p_off(pr, h):
            return (h % 4) * 384 + (h // 4) * 192 + pr * 48

        P = {}
        for orient in ("k", "q"):
            lhs, rhs = (qT, kT) if orient == "q" else (kT, qT)
            dst = sb.tile([128, 4 * 384], bf, tag=f"P{orient}")
            for t in range(2):
                p = pstile()
                for b_ in range(2):
                    g = 2 * t + b_
                    for hh in range(2):
                        h = hh * 4 + g
                        for pr in range(NPAIR):
                            for j in range(2):
                                bb = pr * 2 + j
                                nc.tensor.matmul(
                                    p[j * 64:j * 64 + N, b_,
                                      hh * 192 + pr * 48:hh * 192 + pr * 48 + N],
                                    head_slice(lhs, h, bb),
                                    head_slice(rhs, h, bb),
                                    start=True, stop=True,
                                    tile_position=(g * 32, j * 64),
                                )
                nc.scalar.activation(
                    out=dst[:, 2 * t * 384:(2 * t + 2) * 384]
                        .rearrange("p (b f) -> p b f", f=384),
                    in_=p[:, :, 0:384], func=AF.Exp,
                )
            P[orient] = dst
        Pq, Pk = P["q"], P["k"]

        # ---- denom + w  /  S + SmT ----
        # Per pair one psum slot: bank 0 = den, bank 1 = S. The two j-halves
        # write the same bank, so the j=1 matmuls carry a sync edge on the
        # j=0 group (row-tiled writes to one bank must not overlap in time);
        # this halves the downstream eviction volume.
        if phase_limit < 4:
            _finish(nc, out_d, Pq[:, 0:768].rearrange("p (a f) -> p a f", f=384), NT)
            raise _Stop
        wT = sb.tile([128, NPAIR * 384], bf, tag="wT")
        SmT = sb.tile([128, NPAIR * 384], bf, tag="SmT")

        def adj_qslice(pr):
            """Af[part, h(bcast), t] for one pair."""
            return adjst[:, pr * N:(pr + 1) * N][:, None, :] \
                .to_broadcast((128, H, N))

        def serial_rowgroups(mms_j0, mms_j1):
            for i1 in mms_j1:
                for i0 in mms_j0:
                    add_dep_helper(i1.ins, i0.ins, sync=True,
                                   reason="same-bank row-group serialization")

        dsslot = {}
        for pr in range(NPAIR):
            dp = pstile()
            dsslot[pr] = dp
            groups = [[], []]
            for j in range(2):
                for h in range(H):
                    groups[j].append(nc.tensor.matmul(
                        dp[j * 64:j * 64 + N, 0, h * N:(h + 1) * N],
                        Pk[j * 64:j * 64 + N, p_off(pr, h):p_off(pr, h) + N],
                        adjst[j * 64:j * 64 + N, pr * N:(pr + 1) * N],
                        start=True, stop=True,
                    ))
            serial_rowgroups(groups[0], groups[1])
            rec = sb.tile([128, 384], bf, tag=f"rec{pr}")
            with nc.allow_low_precision(reason="bf16 attn weights; psum accum stays fp32"):
                nc.vector.reciprocal(out=rec[:, :], in_=dp[:, 0, 0:384])
            nc.gpsimd.tensor_tensor(
                out=wT[:, pr * 384:(pr + 1) * 384]
                    .rearrange("p (h t) -> p h t", t=N),
                in0=adj_qslice(pr),
                in1=rec[:, :].rearrange("p (h t) -> p h t", t=N),
                op=OP.mult,
            )
        if phase_limit < 5:
            _finish(nc, out_d, wT[:, 0:768].rearrange("p (a f) -> p a f", f=384), NT)
            raise _Stop
        for pr in range(NPAIR):
            sp = dsslot[pr]
            groups = [[], []]
            for j in range(2):
                for h in range(H):
                    groups[j].append(nc.tensor.matmul(
                        sp[j * 64:j * 64 + N, 1, h * N:(h + 1) * N],
                        Pq[j * 64:j * 64 + N, p_off(pr, h):p_off(pr, h) + N],
                        wT[j * 64:j * 64 + N,
                           pr * 384 + h * N:pr * 384 + h * N + N],
                        start=True, stop=True,
                    ))
            serial_rowgroups(groups[0], groups[1])
            nc.vector.tensor_tensor(
                out=SmT[:, pr * 384:(pr + 1) * 384]
                    .rearrange("p (h t) -> p h t", t=N),
                in0=sp[:, 1, 0:384].rearrange("p (h t) -> p h t", t=N),
                in1=adj_qslice(pr),
                op=OP.mult,
            )

        # ---- cnt / r / r2 / rc: independent of the attention chain ----
        cp = pstile()
        nc.tensor.matmul(cp[:, 0, 0:NT], ones[:, :], adjfl[:, :],
                         start=True, stop=True)
        r = sb.tile([128, NT], f32, tag="r")
        nc.vector.reciprocal(out=r[:, :], in_=cp[:, 0, 0:NT])
        r2 = sb.tile([128, NT], f32, tag="r2")
        nc.gpsimd.tensor_tensor(out=r2[:, :], in0=r[:, :], in1=r[:, :], op=OP.mult)
        rc = sb.tile([128, 2, NT], bf, tag="rc")
        for ot in range(2):
            nc.scalar.activation(
                out=rc[:, ot, :], in_=r[:, :], func=AF.Identity,
                scale=c0[:, ot:ot + 1], bias=bm[:, ot:ot + 1],
            )

        # ---- G: pooled-pre, feature-major; bank = batch parity (= row grp j)
        if phase_limit < 6:
            _finish(nc, out_d, SmT[:, 0:768].rearrange("p (a f) -> p a f", f=384), NT)
            raise _Stop
        gp = pstile()
        for j in range(2):
            for b2 in range(NPAIR):
                bb = b2 * 2 + j
                pr = bb // 2
                for h in range(H):
                    nc.tensor.matmul(
                        gp[(h % 4) * 32:(h % 4) * 32 + 32, j,
                           (h // 4) * 192 + b2 * 48:(h // 4) * 192 + b2 * 48 + N],
                        v[j * 64:j * 64 + N, pr // 2, pr % 2, h * 32:(h + 1) * 32],
                        SmT[j * 64:j * 64 + N,
                            pr * 384 + h * N:pr * 384 + h * N + N],
                        start=True, stop=True,
                        tile_position=(j * 64, (h % 4) * 32),
                    )
        G = sb.tile([128, 2, NT], bf, tag="G")
        for bk in range(2):
            nc.vector.tensor_copy(
                out=G[:, :, :].rearrange("p c (b2 t) -> p c b2 t", t=2 * N)
                    [:, :, :, bk * N:(bk + 1) * N],
                in_=gp[:, bk, 0:384].rearrange("p (c b2 t) -> p c b2 t", c=2, t=N),
            )

        # ---- M2 = (Wm @ Wo)^T-contraction @ G  (Wo/Wm fused host-side;
        # the r2/t column scaling commutes through Wm) ----
        if phase_limit < 7:
            _finish(nc, out_d, G, NT); raise _Stop
        m2 = pstile()
        for ot in range(2):
            for kc in range(2):
                nc.tensor.matmul(
                    m2[:, ot, 0:NT],
                    W["womT"][kc][:, ot * 128:(ot + 1) * 128],
                    G[:, kc, :],
                    start=(kc == 0), stop=(kc == 1),
                )

        # ---- tail: out = M2*r2 + (c0*r + bm) ----
        fin = sb.tile([128, 2, NT], bf, tag="fin")
        osb = sb.tile([128, 2, NT], bf, tag="osb")
        for ot in range(2):
            nc.vector.tensor_tensor(
                out=fin[:, ot, :], in0=m2[:, ot, 0:NT],
                in1=r2[:, :], op=OP.mult,
            )
            nc.vector.tensor_tensor(out=osb[:, ot, :], in0=fin[:, ot, :],
                                    in1=rc[:, ot, :], op=OP.add)
            nc.sync.dma_start(out=out_d.ap()[:, ot * NT:(ot + 1) * NT],
                              in_=osb[:, ot, :])
      except _Stop:
        pass

    nc.compile()
    return nc


def _get_program():
    global _cached
    if _cached is None:
        _cached = _build_program()
    return _cached


def _prep_core_inputs(x_src, adj, Wq, bq, Wk, bk, Wv, bv, Wo, bo, Wm, bm):
    """Host-side shard prep for one core: 8 batches of one direction.
    Matmul-side tensors are cast to bfloat16 (PSUM accumulation stays fp32;
    the reference's own fp32 noise dominates the resulting error)."""
    import ml_dtypes
    f32 = np.float32
    bf = ml_dtypes.bfloat16
    xT = np.ascontiguousarray(np.transpose(x_src, (2, 0, 1)).reshape(FEA, NB * N)).astype(bf)
    Af = (adj > 0).astype(f32)                       # [NB, 48(k), 48(t)]
    adj_st = np.zeros((128, NPAIR * N), f32)
    for p in range(NPAIR):
        adj_st[0:N, p * N:(p + 1) * N] = Af[2 * p]
        adj_st[64:64 + N, p * N:(p + 1) * N] = Af[2 * p + 1]
    adj_flat = np.ascontiguousarray(np.transpose(Af, (1, 0, 2)).reshape(N, NB * N))
    s = 1.0 / np.sqrt(np.float32(DH))
    c0 = (Wm @ bo).astype(f32)
    consts = np.zeros((128, 8 + FEA), f32)
    consts[:, 0:2] = (bq * s).reshape(2, 128).T
    consts[:, 2:4] = bk.reshape(2, 128).T
    consts[:, 4:6] = c0.reshape(2, 128).T
    consts[:, 6:8] = bm.reshape(2, 128).T
    consts[:, 8:] = np.tile(bv[None, :].astype(f32), (128, 1))
    hot = np.concatenate([
        xT,
        np.ascontiguousarray(Wq.T * s).astype(bf),
        np.ascontiguousarray(Wk.T).astype(bf),
        np.ascontiguousarray(Wv.T).astype(bf),
    ], axis=1)
    return {
        "hot": np.ascontiguousarray(hot),
        "adj_st": adj_st.astype(bf),
        "adj_flat": adj_flat.astype(bf),
        "womT": np.ascontiguousarray((Wm @ Wo).T).astype(bf),
        "consts": consts,
    }


def _postprocess_core(out_dev, Af, fallback):
    """out_dev [128, 768] -> mapped [8, 48, 256]; apply fallback select."""
    arr = out_dev.reshape(128, 2, NB, N)
    mapped = np.ascontiguousarray(np.transpose(arr, (2, 3, 1, 0))).reshape(NB, N, FEA)
    cnt = Af.sum(axis=1)                              # [NB, 48(t)]
    return np.where((cnt > 0)[:, :, None], mapped, fallback)


def _make_in_maps(a):
    in_maps, meta = [], []
    for core in range(NCORES):
        dirn = "a" if core < 4 else "s"
        g = core % 4
        bs = slice(g * NB, (g + 1) * NB)
        if dirn == "a":
            x_src, adj, fb = a["sync_fea"][bs], a["sync_adj"][bs], a["async_fea"][bs]
        else:
            x_src, adj, fb = a["async_fea"][bs], a["async_adj"][bs], a["sync_fea"][bs]
        wkeys = [f"{dirn}_{w}" for w in
                 ("Wq", "bq", "Wk", "bk", "Wv", "bv", "Wo", "bo", "Wm", "bm")]
        in_maps.append(_prep_core_inputs(x_src, adj, *[a[k] for k in wkeys]))
        meta.append(((adj > 0).astype(np.float32), fb))
    return in_maps, meta


def _assemble(a, meta, results):
    out = np.zeros((B, N, 4 * FEA), np.float32)
    out[:, :, 2 * FEA:3 * FEA] = a["async_fea"]
    out[:, :, 3 * FEA:] = a["sync_fea"]
    for core in range(NCORES):
        Af, fb = meta[core]
        refined = _postprocess_core(results[core]["outT"], Af, fb)
        g = core % 4
        bs = slice(g * NB, (g + 1) * NB)
        col = slice(0, FEA) if core < 4 else slice(FEA, 2 * FEA)
        out[bs, :, col] = refined
    return out


def kernel(**inputs):
    from concourse import bass_utils

    nc = _get_program()
    a = {k: np.asarray(v) for k, v in inputs.items()}
    in_maps, meta = _make_in_maps(a)
    res = bass_utils.run_bass_kernel_spmd(nc, in_maps, core_ids=list(range(NCORES)))
    return _assemble(a, meta, res.results)



# revision 6
# speedup vs baseline: 1.1784x; 1.1784x over previous
"""Trainium2 Bass kernel for nn_Aggregate (gnn_message_passing).

Sharding: 8 cores = 2 directions x 4 batch-groups. Cores 0-3 compute
refined_async (source = sync_fea, adj = sync_adj, weights a_*) for 8
batches each; cores 4-7 compute refined_sync. The feature passthrough
(output channels 512:1024) and the no-neighbor fallback select are pure
input data movement, done host-side during unsharding.

Device algorithm per core (8 batches, one direction):
  Activations stay feature-major ([feat, node]); batches pair-stacked on
  partitions (rows 0-47 / 64-111) for the per-(batch,head) 48x48 blocks.

  Projections and the output map run as fp8e4 DoubleRow matmuls (0.5
  cycles/col, the full 256-deep contraction in one instruction): x and
  all weights are stored [128, 2, *] with the k-chunk in dim 1, weights
  host-prescaled by 32 (64 for Wm@Wo) to sit in e4m3's mantissa sweet
  spot; the descale rides existing eviction scale slots.

  Two exact algebraic folds shrink both data and compute:
   - bk is dropped: q^T bk and bq^T bk are per-query-constant in the
     softmax over keys, so they cancel; only (x Wq + bq)^T (x Wk) is
     needed.
   - bv is folded: sum_k SmT_h[k,t] = cnt[t] for every head, so the
     v-bias contributes (Wm(Wo bv))*r to the output; it merges with the
     existing (Wm bo)*r term into c0 = Wm(Wo bv + bo).

    qT = (1/32)*q_psum + s*bq   (Act)     kT = (1/32)*k_psum  (DVE)
    per (batch, head):  Pq = exp(qT_h^T kT_h), Pk = exp(kT_h^T qT_h)
                        den = Pk^T Af ; rec = 1/den ; w = Af * rec
                        ST  = Pq^T w ; SmT = ST * Af ; G_h = v_h^T SmT
    G' = gp * (16 r^2)  [fp8]  ;  M2 = (64 WmWo)^T G'  (DoubleRow)
    out = M2*(1/1024) + (c0*r + bm)

Built on bacc.Bacc: its compile() legalizes sync waits (TRN2 allows one
wait per instruction) via ldweights-wait motion + event semaphores.
"""

import numpy as np

FEA, H, B, N = 256, 8, 32, 48
DH = FEA // H
NB = 8            # batches per core
NPAIR = NB // 2
NCORES = 8
NT = NB * N       # 384

WS = 32.0         # fp8 prescale for Wq/Wk/Wv
WOMS = 64.0       # fp8 prescale for Wm@Wo
ALPHA = 16.0      # G' = gp * r^2 * ALPHA; out = m2/(WOMS*ALPHA) + rc

_cached = None


class _Stop(Exception):
    pass


def _build_program(phase_limit=99):
    import concourse.tile as tile
    from concourse.tile import add_dep_helper
    from concourse import bacc, mybir
    from contextlib import ExitStack

    f32 = mybir.dt.float32
    bf = mybir.dt.bfloat16
    f8 = mybir.dt.float8e4
    AF = mybir.ActivationFunctionType
    OP = mybir.AluOpType
    DR = mybir.MatmulPerfMode.DoubleRow

    nc = bacc.Bacc("TRN2", target_bir_lowering=False, debug=False)

    # ---- DRAM I/O ----
    hot_d = nc.dram_tensor("hot", [128, 2, NT + 3 * FEA], f8, kind="ExternalInput")
    wom_d = nc.dram_tensor("wom", [128, 2, FEA], f8, kind="ExternalInput")
    adjt_d = nc.dram_tensor("adjt", [128, 584], bf, kind="ExternalInput")
    out_d = nc.dram_tensor("outT", [128, 2 * NT], bf, kind="ExternalOutput")

    with ExitStack() as ctx:
      try:
        tc = ctx.enter_context(tile.TileContext(nc))
        sb = ctx.enter_context(tc.tile_pool(name="sb", bufs=1))
        ps = ctx.enter_context(tc.tile_pool(name="ps", bufs=4, space="PSUM"))

        # ---- loads: 3 DMAs, one per engine queue so SEQ portions overlap;
        # HWDGE issue (shared, ~625ns each) pipelines with the transfers ----
        hot = sb.tile([128, 2, NT + 3 * FEA], f8, tag="hot")
        nc.sync.dma_start(out=hot[:, :, :], in_=hot_d.ap()[:, :, :])
        adjt = sb.tile([128, 584], bf, tag="adjt")
        nc.scalar.dma_start(out=adjt[:, :], in_=adjt_d.ap()[:, :])
        wom = sb.tile([128, 2, FEA], f8, tag="wom")
        nc.sync.dma_start(out=wom[:, :, :], in_=wom_d.ap()[:, :, :])

        xT = hot[:, :, 0:NT]
        wq = hot[:, :, NT:NT + FEA]
        wk = hot[:, :, NT + FEA:NT + 2 * FEA]
        wv = hot[:, :, NT + 2 * FEA:NT + 3 * FEA]
        adjst = adjt[:, 0:NPAIR * N]
        bqs = adjt[:, 192:194]
        c0 = adjt[:, 194:196]
        bmc = adjt[:, 196:198]
        adjfl = adjt[0:N, 200:584]

        ones = sb.tile([N, 128], bf, tag="ones")
        nc.gpsimd.memset(ones[:, :], 1.0)

        _psn = [0]

        def pstile():
            _psn[0] += 1
            return ps.tile([128, 2, 512], f32, tag="ps", name=f"ps{_psn[0]}")

        # ---- q/k projections: fp8 DoubleRow, full 256-contraction per mm.
        # rhs free is kept <= 512 (2 k-tiles x 192), so 4 mm per tensor. ----
        pq, pk = pstile(), pstile()
        for p, w_ in ((pq, wq), (pk, wk)):
            for ot in range(2):
                for hf in range(2):
                    nc.tensor.matmul(
                        p[:, ot, hf * 192:(hf + 1) * 192],
                        w_[:, :, ot * 128:(ot + 1) * 128],
                        xT[:, :, hf * 192:(hf + 1) * 192],
                        start=True, stop=True, perf_mode=DR,
                    )

        # ---- cnt / r / r2s / rc: independent of the attention chain ----
        cp = pstile()
        nc.tensor.matmul(cp[:, 0, 0:NT], ones[:, :], adjfl[:, :],
                         start=True, stop=True)

        # ---- v: node-major, fp8 DoubleRow (one mm per batch) ----
        pv = pstile()
        for b in range(NB):
            pr, j = b // 2, b % 2
            bank, off = pr // 2, (pr % 2) * FEA
            nc.tensor.matmul(
                pv[j * 64:j * 64 + N, bank, off:off + FEA],
                xT[:, :, b * N:(b + 1) * N],
                wv[:, :, :],
                start=True, stop=True, perf_mode=DR,
            )

        # ---- evictions: q on Act (scale + per-ot bias), k and v on DVE ----
        qT = sb.tile([128, 2, NT], bf, tag="qT")
        for ot in range(2):
            nc.scalar.activation(
                out=qT[:, ot, :], in_=pq[:, ot, 0:NT], func=AF.Identity,
                scale=1.0 / WS, bias=bqs[:, ot:ot + 1],
            )
        kT = sb.tile([128, 2, NT], bf, tag="kT")
        with nc.allow_low_precision(reason="bf16 activations"):
            nc.vector.tensor_scalar_mul(
                out=kT[:, :, :], in0=pk[:, :, 0:NT], scalar1=1.0 / WS,
            )

        r = sb.tile([128, NT], f32, tag="r")
        nc.vector.reciprocal(out=r[:, :], in_=cp[:, 0, 0:NT])

        v = sb.tile([128, 2, 2, FEA], bf, tag="v")
        with nc.allow_low_precision(reason="bf16 activations"):
            nc.vector.tensor_scalar_mul(
                out=v[:, :, :, :],
                in0=pv[:, :, :].rearrange("p a (c o) -> p a c o", o=FEA),
                scalar1=1.0 / WS,
            )
        # r2s = 16*r^2 (the 1/64 wom prescale and 1/16 headroom are repaid
        # in the final 1/1024); rc = c0*r + bm
        r2s = sb.tile([128, NT], bf, tag="r2s")
        with nc.allow_low_precision(reason="bf16 scale vector"):
            nc.vector.scalar_tensor_tensor(
                out=r2s[:, :], in0=r[:, :], scalar=ALPHA, in1=r[:, :],
                op0=OP.mult, op1=OP.mult,
            )

        def head_slice(t, h, b):
            """[32, 48] slice of a feature-major [128, 2, NT] tile."""
            return t[(h % 4) * 32:(h % 4) * 32 + 32, h // 4, b * N:(b + 1) * N]

        # ---- scores, both orientations, exp ----
        # Row-tiled matmuls must not write the same PSUM bank concurrently
        # (HW constraint). Map row-group -> bank bijectively: tile t's bank b
        # holds head-group g = 2t+b (heads {g, g+4}), pairs in the free dim.
        if phase_limit < 3:
            _finish(nc, out_d, qT); raise _Stop

        def p_off(pr, h):
            return (h % 4) * 384 + (h // 4) * 192 + pr * 48

        P = {}
        for orient in ("k", "q"):
            lhs, rhs = (qT, kT) if orient == "q" else (kT, qT)
            dst = sb.tile([128, 4 * 384], bf, tag=f"P{orient}")
            for t in range(2):
                p = pstile()
                for b_ in range(2):
                    g = 2 * t + b_
                    for hh in range(2):
                        h = hh * 4 + g
                        for pr in range(NPAIR):
                            for j in range(2):
                                bb = pr * 2 + j
                                nc.tensor.matmul(
                                    p[j * 64:j * 64 + N, b_,
                                      hh * 192 + pr * 48:hh * 192 + pr * 48 + N],
                                    head_slice(lhs, h, bb),
                                    head_slice(rhs, h, bb),
                                    start=True, stop=True,
                                    tile_position=(g * 32, j * 64),
                                )
                nc.scalar.activation(
                    out=dst[:, 2 * t * 384:(2 * t + 2) * 384]
                        .rearrange("p (b f) -> p b f", f=384),
                    in_=p[:, :, 0:384], func=AF.Exp,
                )
            P[orient] = dst
        Pq, Pk = P["q"], P["k"]

        # rc emitted after the exps so it doesn't delay them on Act
        rc = sb.tile([128, 2, NT], bf, tag="rc")
        for ot in range(2):
            nc.scalar.activation(
                out=rc[:, ot, :], in_=r[:, :], func=AF.Identity,
                scale=c0[:, ot:ot + 1], bias=bmc[:, ot:ot + 1],
            )

        # ---- den + w ----
        # Per pair one psum slot: bank 0 = den, bank 1 = S. The two j-halves
        # write the same bank, so the j=1 matmuls carry a sync edge on the
        # j=0 group (row-tiled writes to one bank must not overlap in time).
        if phase_limit < 4:
            _finish(nc, out_d, Pq[:, 0:768].rearrange("p (a f) -> p a f", f=384))
            raise _Stop
        wT = sb.tile([128, NPAIR * 384], bf, tag="wT")
        SmT = sb.tile([128, NPAIR * 384], bf, tag="SmT")

        def adj_qslice(pr):
            """Af[part, h(bcast), t] for one pair."""
            return adjst[:, pr * N:(pr + 1) * N][:, None, :] \
                .to_broadcast((128, H, N))

        def serial_rowgroups(mms_j0, mms_j1):
            for i1 in mms_j1:
                for i0 in mms_j0:
                    add_dep_helper(i1.ins, i0.ins, sync=True,
                                   reason="same-bank row-group serialization")

        dsslot = {}
        for pr in range(NPAIR):
            dp = pstile()
            dsslot[pr] = dp
            groups = [[], []]
            for j in range(2):
                for h in range(H):
                    groups[j].append(nc.tensor.matmul(
                        dp[j * 64:j * 64 + N, 0, h * N:(h + 1) * N],
                        Pk[j * 64:j * 64 + N, p_off(pr, h):p_off(pr, h) + N],
                        adjst[j * 64:j * 64 + N, pr * N:(pr + 1) * N],
                        start=True, stop=True,
                    ))
            serial_rowgroups(groups[0], groups[1])
            rec = sb.tile([128, 384], bf, tag=f"rec{pr}")
            with nc.allow_low_precision(reason="bf16 attn weights; psum accum stays fp32"):
                nc.vector.reciprocal(out=rec[:, :], in_=dp[:, 0, 0:384])
            # all-bf16 all-SBUF: runs in the DVE 2x/4x fast path
            nc.vector.tensor_tensor(
                out=wT[:, pr * 384:(pr + 1) * 384]
                    .rearrange("p (h t) -> p h t", t=N),
                in0=adj_qslice(pr),
                in1=rec[:, :].rearrange("p (h t) -> p h t", t=N),
                op=OP.mult,
            )
        if phase_limit < 5:
            _finish(nc, out_d, wT[:, 0:768].rearrange("p (a f) -> p a f", f=384))
            raise _Stop

        # ---- S + SmT (SmT split DVE/Pool to pipeline the 4-pair wave) ----
        smt_insts = []
        for pr in range(NPAIR):
            sp = dsslot[pr]
            groups = [[], []]
            for j in range(2):
                for h in range(H):
                    groups[j].append(nc.tensor.matmul(
                        sp[j * 64:j * 64 + N, 1, h * N:(h + 1) * N],
                        Pq[j * 64:j * 64 + N, p_off(pr, h):p_off(pr, h) + N],
                        wT[j * 64:j * 64 + N,
                           pr * 384 + h * N:pr * 384 + h * N + N],
                        start=True, stop=True,
                    ))
            serial_rowgroups(groups[0], groups[1])
            eng = nc.gpsimd if pr % 2 == 0 else nc.vector
            smt_insts.append(eng.tensor_tensor(
                out=SmT[:, pr * 384:(pr + 1) * 384]
                    .rearrange("p (h t) -> p h t", t=N),
                in0=sp[:, 1, 0:384].rearrange("p (h t) -> p h t", t=N),
                in1=adj_qslice(pr),
                op=OP.mult,
            ))

        # ---- G: pooled-pre, feature-major; bank = batch parity (= row grp j)
        if phase_limit < 6:
            _finish(nc, out_d, SmT[:, 0:768].rearrange("p (a f) -> p a f", f=384))
            raise _Stop
        gp = pstile()
        G = sb.tile([128, 2, NT], f8, tag="G")
        for b2 in range(NPAIR):
            for j in range(2):
                bb = b2 * 2 + j
                pr = bb // 2
                for h in range(H):
                    nc.tensor.matmul(
                        gp[(h % 4) * 32:(h % 4) * 32 + 32, j,
                           (h // 4) * 192 + b2 * 48:(h // 4) * 192 + b2 * 48 + N],
                        v[j * 64:j * 64 + N, pr // 2, pr % 2, h * 32:(h + 1) * 32],
                        SmT[j * 64:j * 64 + N,
                            pr * 384 + h * N:pr * 384 + h * N + N],
                        start=True, stop=True,
                        tile_position=(j * 64, (h % 4) * 32),
                    )
            # evict pair b2 as soon as its 16 matmuls land; fold in r2s
            eng = nc.vector if b2 % 2 == 0 else nc.gpsimd
            with nc.allow_low_precision(reason="fp8 G; error repaid in 256-contraction"):
                eng.tensor_tensor(
                    out=G[:, :, :].rearrange("p c (b2 j n) -> p c b2 j n",
                                             b2=NPAIR, j=2)[:, :, b2, :, :],
                    in0=gp[:, :, 0:384].rearrange("p j (c b2 n) -> p c b2 j n",
                                                  c=2, b2=NPAIR)[:, :, b2, :, :],
                    in1=r2s[:, :].rearrange("p (b2 j n) -> p b2 j n",
                                            b2=NPAIR, j=2)[:, b2, :, :]
                        [:, None, :, :].to_broadcast((128, 2, 2, N)),
                    op=OP.mult,
                )

        # ---- M2 = (64 WmWo)^T G'  (fp8 DoubleRow) ----
        if phase_limit < 7:
            _finish(nc, out_d, G); raise _Stop
        m2 = pstile()
        for ot in range(2):
            for hf in range(2):
                nc.tensor.matmul(
                    m2[:, ot, hf * 192:(hf + 1) * 192],
                    wom[:, :, ot * 128:(ot + 1) * 128],
                    G[:, :, hf * 192:(hf + 1) * 192],
                    start=True, stop=True, perf_mode=DR,
                )

        # ---- tail: out = m2/1024 + rc, split DVE/Pool, DMA per ot ----
        osb = sb.tile([128, 2, NT], bf, tag="osb")
        for ot in range(2):
            eng = nc.vector if ot == 0 else nc.gpsimd
            eng.scalar_tensor_tensor(
                out=osb[:, ot, :], in0=m2[:, ot, 0:NT],
                scalar=1.0 / (WOMS * ALPHA), in1=rc[:, ot, :],
                op0=OP.mult, op1=OP.add,
            )
            deng = nc.sync if ot == 0 else nc.scalar
            deng.dma_start(out=out_d.ap()[:, ot * NT:(ot + 1) * NT],
                           in_=osb[:, ot, :])
      except _Stop:
        pass

    nc.compile()
    return nc


def _finish(nc, out_d, tile_ap):
    ap = tile_ap[:, :, :].rearrange("p a t -> p (a t)")
    for f0 in range(0, 2 * NT, 96):
        nc.sync.dma_start(out=out_d.ap()[:, f0:f0 + 96], in_=ap[:, f0:f0 + 96])


def _get_program():
    global _cached
    if _cached is None:
        _cached = _build_program()
    return _cached


def _prep_core_inputs(x_src, adj, Wq, bq, Wk, bk, Wv, bv, Wo, bo, Wm, bm):
    """Host-side shard prep for one core: 8 batches of one direction.
    Matmul-side tensors go to fp8e4 (weights prescaled into e4m3's sweet
    spot; descales folded into on-chip eviction constants)."""
    import ml_dtypes
    f32 = np.float32
    bf = ml_dtypes.bfloat16
    f8 = ml_dtypes.float8_e4m3
    s = 1.0 / np.sqrt(np.float32(DH))

    xT = np.transpose(x_src, (2, 0, 1)).reshape(FEA, NT)
    hot = np.zeros((128, 2, NT + 3 * FEA), f32)
    for kc in range(2):
        rows = slice(kc * 128, (kc + 1) * 128)
        hot[:, kc, 0:NT] = xT[rows]
        hot[:, kc, NT:NT + FEA] = (WS * s) * Wq.T[rows]
        hot[:, kc, NT + FEA:NT + 2 * FEA] = WS * Wk.T[rows]
        hot[:, kc, NT + 2 * FEA:NT + 3 * FEA] = WS * Wv.T[rows]

    womT = (Wm @ Wo).T
    wom = np.zeros((128, 2, FEA), f32)
    for kc in range(2):
        wom[:, kc, :] = WOMS * womT[kc * 128:(kc + 1) * 128]

    Af = (adj > 0).astype(f32)                       # [NB, 48(k), 48(t)]
    adjt = np.zeros((128, 584), f32)
    for p in range(NPAIR):
        adjt[0:N, p * N:(p + 1) * N] = Af[2 * p]
        adjt[64:64 + N, p * N:(p + 1) * N] = Af[2 * p + 1]
    adjt[:, 192:194] = (bq * s).reshape(2, 128).T
    c0 = (Wm @ (Wo @ bv + bo)).astype(f32)
    adjt[:, 194:196] = c0.reshape(2, 128).T
    adjt[:, 196:198] = bm.reshape(2, 128).T
    adjt[0:N, 200:584] = np.transpose(Af, (1, 0, 2)).reshape(N, NT)
    return {
        "hot": np.ascontiguousarray(hot).astype(f8),
        "wom": np.ascontiguousarray(wom).astype(f8),
        "adjt": adjt.astype(bf),
    }


def _postprocess_core(out_dev, Af, fallback):
    """out_dev [128, 768] -> mapped [8, 48, 256]; apply fallback select."""
    arr = out_dev.reshape(128, 2, NB, N)
    mapped = np.ascontiguousarray(np.transpose(arr, (2, 3, 1, 0))).reshape(NB, N, FEA)
    cnt = Af.sum(axis=1)                              # [NB, 48(t)]
    return np.where((cnt > 0)[:, :, None], mapped, fallback)


def _make_in_maps(a):
    in_maps, meta = [], []
    for core in range(NCORES):
        dirn = "a" if core < 4 else "s"
        g = core % 4
        bs = slice(g * NB, (g + 1) * NB)
        if dirn == "a":
            x_src, adj, fb = a["sync_fea"][bs], a["sync_adj"][bs], a["async_fea"][bs]
        else:
            x_src, adj, fb = a["async_fea"][bs], a["async_adj"][bs], a["sync_fea"][bs]
        wkeys = [f"{dirn}_{w}" for w in
                 ("Wq", "bq", "Wk", "bk", "Wv", "bv", "Wo", "bo", "Wm", "bm")]
        in_maps.append(_prep_core_inputs(x_src, adj, *[a[k] for k in wkeys]))
        meta.append(((adj > 0).astype(np.float32), fb))
    return in_maps, meta


def _assemble(a, meta, results):
    out = np.zeros((B, N, 4 * FEA), np.float32)
    out[:, :, 2 * FEA:3 * FEA] = a["async_fea"]
    out[:, :, 3 * FEA:] = a["sync_fea"]
    for core in range(NCORES):
        Af, fb = meta[core]
        refined = _postprocess_core(results[core]["outT"], Af, fb)
        g = core % 4
        bs = slice(g * NB, (g + 1) * NB)
        col = slice(0, FEA) if core < 4 else slice(FEA, 2 * FEA)
        out[bs, :, col] = refined
    return out


def kernel(**inputs):
    from concourse import bass_utils

    nc = _get_program()
    a = {k: np.asarray(v) for k, v in inputs.items()}
    in_maps, meta = _make_in_maps(a)
    res = bass_utils.run_bass_kernel_spmd(nc, in_maps, core_ids=list(range(NCORES)))
    return _assemble(a, meta, res.results)
